# revision 5
# baseline (speedup 1.0000x reference)
"""Trainium2 Bass kernel for nn_CrossAttention_28183575396415.

The reference block-mask gives every query exactly one key (kv = q_idx // 3),
so the softmax weight is identically 1 and the q/k projections, RMSNorm and
RoPE are dead code.  The module reduces to

    out[b, t] = x_kv[b, t // 3] @ Wv.T @ Wproj.T
              = x_kv[b, t // 3] @ WfT          with WfT = Wv.T @ Wproj.T

Strategy (8 NeuronCores, SPMD):
  - Host folds the two projection matrices into WfT (computed in float64).
  - The 4*2048 = 8192 kv rows are row-sharded 8 ways (1024 rows/core).
    Each core's shard is pre-transposed on host so every device DMA is a
    natural contiguous load; the shard and the weight are concatenated into
    one [1024(k), 2048] bf16 input:
        xw[:, :1024]  = x_shard.T   (k on partitions = contraction dim)
        xw[:, 1024:]  = WfT
  - Schedule (timings from the 46.1us baseline trace):
    * ~11 warm-up matmuls on a memset scratch tile run while the input
      streams in, so the PE_HAM clock gate reaches 8/8 (2.4 GHz) before the
      first real matmul - the baseline paid ~1.7us of 1.2 GHz cold matmuls.
    * k-tile 0 is split across both HWDGE rings (x part on sync, W part on
      scalar) so the first real matmul's 384KB dependency becomes two
      concurrent 256/128KB transfers and the PE starts ~1us sooner.
    * Pass 0 (cc half 0, rows m0-m5) runs k-major in lockstep with the
      input stream; at the last k-tile each row's eviction + 3x-replicated
      output store issues immediately, so the output DMA stream starts
      right as the input stream ends (the baseline's all-rows k-major pass
      started stores 11us later, leaving a 10.8us un-overlapped tail).
    * Bridge (cc0, m6-m7) + pass 1 (cc1, all rows) run m-major, one tile
      retiring every ~1.7us, keeping the output ring saturated to the end.
  - Each z tile is written to HBM with a single DMA whose source AP repeats
    the tile 3x (stride-0 middle dim) - the t//3 replication - giving this
    core's contiguous [3072, 1024] slice of the flattened output in bf16.
  - Host unshard = concatenate the 8 slices and upcast to float32.
"""

import json
import os

import numpy as np

import concourse.bass as bass
import concourse.mybir as mybir
from bass_rust import AP
from concourse.tile import TileContext
from concourse.vector_clock import ScopedClock
from concourse.bass_utils import run_bass_kernel_spmd

P = 128          # partitions
C = 1024         # model dim
K_T = C // P     # k tiles
M_T = C // P     # row tiles per core shard
N = 512          # matmul free dim (one PSUM bank of fp32)
L = 3            # replication factor (Tq // Tkv)
ROWS_PER_CORE = 1024
N_CORES = 8
W1 = ROWS_PER_CORE + N   # x | W-cc0 columns per k-tile row block

# compute dtype: "bf16" (half the input DMA), "f32r"/"f32" for debugging
COMPUTE_DT = os.environ.get("KERNEL_COMPUTE_DT", "bf16")
# output dtype on device: "bf16" (host upcasts) or "f32"
OUT_DT = os.environ.get("KERNEL_OUT_DT", "bf16")
# PE warm-up matmuls issued before the first real matmul
N_WARM = int(os.environ.get("KERNEL_N_WARM", "20"))
# k-major lead rows in pass 0
M_LEAD = int(os.environ.get("KERNEL_M_LEAD", "6"))


class SlimTailTileContext(TileContext):
    """Tile's kernel tail is drain -> barrier -> per-semaphore clear
    instructions -> barrier.  The clears only matter if the loaded NEFF
    executes more than once; every kernel() call here builds a fresh jit
    executable (fresh NEFF load, semaphores re-initialized), so skip them
    and the second barrier.  The drain still waits for every DMA queue,
    so outputs are complete before the program ends."""

    def _drain_and_barrier(self, tick_clock, wait_clock):
        drain_inst = self.nc.sync.drain()
        wait_clock.add_sem_waits(
            drain_inst.ins, ScopedClock({None: tick_clock.global_clock})
        )
        popped = self.nc._tile_sem_poison_stack.pop()
        assert popped is self._sem_poison


def _split_multiwaits(nc: bass.Bass) -> None:
    """This container's walrus allows only ONE sync-wait on several
    instruction formats (Drain/CTRL, Matmult's LDWEIGHTS half, ...).  Tile
    can emit more.  Post-pass the serialized BIR: for any instruction with
    >1 on_wait, hoist all but the last wait onto single-wait EventSemaphore
    carriers inserted immediately before it on the same engine (waits then
    execute in queue order - semantics unchanged)."""
    raw = bass.Bass.to_json_bytes(nc)
    j = json.loads(raw)
    for f in j["functions"]:
        for bb in f["blocks"]:
            new_insts = []
            for ins in bb["instructions"]:
                si = ins.get("sync_info")
                waits = si.get("on_wait", []) if si else []
                if len(waits) > 1:
                    for i, w in enumerate(waits[:-1]):
                        carrier = {
                            "engine": ins["engine"],
                            "ins": [],
                            "outs": [],
                            "name": f"{ins['name']}_hw{i}",
                            "opcode": "EventSemaphore",
                            "sync_info": {"on_update": [], "on_wait": [w]},
                        }
                        if "debug" in ins:
                            carrier["debug"] = ins["debug"]
                        new_insts.append(carrier)
                    si["on_wait"] = waits[-1:]
                new_insts.append(ins)
            bb["instructions"] = new_insts
    patched = json.dumps(j).encode()
    nc.to_json_bytes = lambda: patched


def _rep3_src(zh_ap):
    """Source AP reading a [P, N] SBUF tile as [P, L, N] via a stride-0
    middle dim - the DMA replicates each row L times."""
    lay = zh_ap.ap
    assert len(lay) == 2, lay
    return AP(tensor=zh_ap.tensor, offset=zh_ap.offset, ap=[lay[0], [0, L], lay[1]])


def _build(compute_dt: str, out_dt: str) -> bass.Bass:
    nc = bass.Bass("TRN2")
    in_mydt = {
        "bf16": mybir.dt.bfloat16,
        "f32r": mybir.dt.float32r,
        "f32": mybir.dt.float32,
    }[compute_dt]
    out_mydt = {"bf16": mybir.dt.bfloat16, "f32": mybir.dt.float32}[out_dt]

    xw = nc.dram_tensor("xw", [C, 2 * C], in_mydt, kind="ExternalInput")
    out = nc.dram_tensor(
        "out", [L * ROWS_PER_CORE, C], out_mydt, kind="ExternalOutput"
    )
    # out row (L*g + r) <- z row g
    out_rep = out.rearrange("(g r) c -> g r c", r=L)  # [1024, L, 1024]

    with SlimTailTileContext(nc) as tc:
        with (
            tc.tile_pool(name="xw", bufs=1) as xw_pool,
            tc.tile_pool(name="psum", bufs=8, space="PSUM") as psum_pool,
            tc.tile_pool(name="zout", bufs=16) as z_pool,
        ):
            # --- scratch for PE warm-up: memset once, matmul garbage-free
            # zeros into a scratch PSUM bank.  Keeps PE_HAM's activity
            # window busy from the end of the engine preamble (~t+0.7us)
            # until the first real matmul (~t+4us) so the real matmuls run
            # at 2.4 GHz.  N=128 warm-ups are ~110ns cold, so over/under-
            # shooting the real-data arrival costs almost nothing.
            warm = xw_pool.tile([P, P], in_mydt, name="warm", tag="warm")
            nc.vector.memset(warm[:], 0.0)

            # --- input stream.  k0 is split so the very first matmul's
            # dependency is only 96KB (x k0 m0-block on sync, first half of
            # W k0 on scalar); k1..k7 alternate rings; the pass-1-only W
            # half (wc1) follows, split across both rings.
            xwk = []
            for k in range(K_T):
                t = xw_pool.tile([P, W1], in_mydt, name=f"xwk{k}", tag=f"xwk{k}")
                xwk.append(t)
            NH = N // 2
            nc.sync.dma_start(xwk[0][:, :P], xw[0:P, :P])
            nc.scalar.dma_start(xwk[0][:, C : C + NH], xw[0:P, C : C + NH])
            nc.sync.dma_start(xwk[0][:, P:C], xw[0:P, P:C])
            nc.scalar.dma_start(xwk[0][:, C + NH : W1], xw[0:P, C + NH : W1])
            in_eng = {1: nc.sync, 3: nc.sync, 5: nc.sync, 7: nc.sync,
                      2: nc.scalar, 4: nc.scalar, 6: nc.scalar}
            for k in range(1, K_T):
                in_eng[k].dma_start(xwk[k][:], xw[k * P : (k + 1) * P, :W1])
            wc1 = xw_pool.tile([P, K_T * N], in_mydt, name="wc1", tag="wc1")
            H = K_T // 2
            nc.sync.dma_start(
                wc1[:, : H * N].rearrange("p (k m) -> p k m", k=H),
                xw[: H * P, W1:].rearrange("(k p) m -> p k m", p=P),
            )
            nc.scalar.dma_start(
                wc1[:, H * N :].rearrange("p (k m) -> p k m", k=H),
                xw[H * P :, W1:].rearrange("(k p) m -> p k m", p=P),
            )

            # --- PE warm-up matmuls (independent of any DMA)
            if N_WARM:
                wps = psum_pool.tile([P, P], mybir.dt.float32, name="wps", tag="ps")
                for i in range(N_WARM):
                    nc.tensor.matmul(
                        wps[:], warm[:], warm[:], start=True, stop=True
                    )

            out_eng = [nc.sync, nc.scalar]
            n_trig = [0]

            def store(zh, m, cc, lo=0, hi=N, eng=None):
                dst = out_rep[m * P : (m + 1) * P, :, cc * N + lo : cc * N + hi]
                src = zh[:, lo:hi]
                if eng is None:
                    eng = out_eng[n_trig[0] % 2]
                    n_trig[0] += 1
                eng.dma_start(dst, _rep3_src(src))

            evict = [nc.vector.tensor_copy, nc.scalar.copy]

            # --- pass 0 (columns 0:512, rows m0..m{M_LEAD-1}): k-major in
            # lockstep with the input stream.  At the last k-tile each row's
            # eviction + store issues immediately after that row's final
            # matmul, so the output stream starts while the PE finishes the
            # remaining rows.
            ps0 = [
                psum_pool.tile([P, N], mybir.dt.float32, name=f"ps0_{m}", tag="ps")
                for m in range(M_LEAD)
            ]
            NH = N // 2
            for k in range(K_T):
                t = xwk[k]
                rhs = t[:, C:W1]
                for m in range(M_LEAD):
                    if k == 0 and m == 0:
                        # split k0/m0 into two N=256 halves: the first real
                        # matmul then only waits on the 32KB x-m0 chunk and
                        # the 64KB first W half.  acc semantics: the first
                        # half clears the bank's has_written bits; the
                        # second half overwrites its (unset) columns.
                        nc.tensor.matmul(
                            ps0[0][:, :NH], t[:, :P], rhs[:, :NH],
                            start=True, stop=False,
                        )
                        nc.tensor.matmul(
                            ps0[0][:, NH:], t[:, :P], rhs[:, NH:],
                            start=False, stop=False, skip_group_check=True,
                        )
                        continue
                    nc.tensor.matmul(
                        ps0[m][:],
                        t[:, m * P : (m + 1) * P],
                        rhs,
                        start=(k == 0),
                        stop=(k == K_T - 1),
                    )
                    if k == K_T - 1:
                        zh = z_pool.tile([P, N], out_mydt, name=f"z0_{m}", tag="z")
                        evict[m % 2](zh[:], ps0[m][:])
                        store(zh, m, 0)

            # --- bridge (columns 0:512, rows m{M_LEAD}..m7): m-major
            for m in range(M_LEAD, M_T):
                ps = psum_pool.tile([P, N], mybir.dt.float32, name=f"ps0_{m}", tag="ps")
                for k in range(K_T):
                    t = xwk[k]
                    nc.tensor.matmul(
                        ps[:],
                        t[:, m * P : (m + 1) * P],
                        t[:, C:W1],
                        start=(k == 0),
                        stop=(k == K_T - 1),
                    )
                zh = z_pool.tile([P, N], out_mydt, name=f"z0_{m}", tag="z")
                evict[m % 2](zh[:], ps[:])
                store(zh, m, 0)

            # --- pass 1 (columns 512:1024): m-major; the final tile is
            # evicted and stored as two halves on parallel engines to
            # shorten the tail.
            for m in range(M_T):
                ps = psum_pool.tile([P, N], mybir.dt.float32, name=f"ps1_{m}", tag="ps")
                for k in range(K_T):
                    t = xwk[k]
                    nc.tensor.matmul(
                        ps[:],
                        t[:, m * P : (m + 1) * P],
                        wc1[:, k * N : (k + 1) * N],
                        start=(k == 0),
                        stop=(k == K_T - 1),
                    )
                zh = z_pool.tile([P, N], out_mydt, name=f"z1_{m}", tag="z")
                if m == M_T - 1:
                    h = N // 2
                    nc.vector.tensor_copy(zh[:, :h], ps[:, :h])
                    nc.scalar.copy(zh[:, h:], ps[:, h:])
                    store(zh, m, 1, 0, h, eng=nc.sync)
                    store(zh, m, 1, h, N, eng=nc.scalar)
                else:
                    evict[m % 2](zh[:], ps[:])
                    store(zh, m, 1)

    _split_multiwaits(nc)
    return nc


_NC_CACHE: dict = {}


def _get_nc(compute_dt: str, out_dt: str) -> bass.Bass:
    key = (compute_dt, out_dt, N_WARM, M_LEAD)
    if key not in _NC_CACHE:
        _NC_CACHE[key] = _build(compute_dt, out_dt)
    return _NC_CACHE[key]


def kernel(x_q, x_kv, Wq, Wk, Wv, Wproj, _compute_dt=None, _out_dt=None):
    compute_dt = _compute_dt or COMPUTE_DT
    out_dt = _out_dt or OUT_DT
    B, Tkv, C_ = x_kv.shape
    assert (B, Tkv, C_) == (4, 2048, C)

    # Fold the two projections: z = x @ Wv.T @ Wproj.T = x @ WfT
    WfT = (Wv.astype(np.float64).T @ Wproj.astype(np.float64).T).astype(np.float32)

    x_flat = x_kv.reshape(B * Tkv, C)
    in_maps = []
    for c in range(N_CORES):
        shard = x_flat[c * ROWS_PER_CORE : (c + 1) * ROWS_PER_CORE]
        xw = np.concatenate([shard.T, WfT], axis=1)  # [C(k), 2048]
        if compute_dt == "bf16":
            import ml_dtypes

            xw = xw.astype(ml_dtypes.bfloat16)
        else:
            xw = np.ascontiguousarray(xw)
        in_maps.append({"xw": xw})

    nc = _get_nc(compute_dt, out_dt)
    res = run_bass_kernel_spmd(nc, in_maps, core_ids=list(range(N_CORES)))

    Tq = L * Tkv
    blocks = [res.results[c]["out"] for c in range(N_CORES)]
    out_flat = np.concatenate(blocks, axis=0)  # [B*Tq, C]
    return out_flat.reshape(B, Tq, C).astype(np.float32)


# revision 8
# speedup vs baseline: 1.0340x; 1.0340x over previous
"""Trainium2 Bass kernel for nn_CrossAttention_28183575396415.

The reference block-mask gives every query exactly one key (kv = q_idx // 3),
so the softmax weight is identically 1 and the q/k projections, RMSNorm and
RoPE are dead code.  The module reduces to

    out[b, t] = x_kv[b, t // 3] @ Wv.T @ Wproj.T
              = x_kv[b, t // 3] @ WfT          with WfT = Wv.T @ Wproj.T

Strategy (8 NeuronCores, SPMD):
  - Host folds the two projection matrices into WfT (computed in float64).
  - The 4*2048 = 8192 kv rows are row-sharded 8 ways (1024 rows/core).
    Each core's shard is pre-transposed on host so every device DMA is a
    natural contiguous load; the shard and the weight are concatenated into
    one [1024(k), 2048] bf16 input:
        xw[:, :1024]  = x_shard.T   (k on partitions = contraction dim)
        xw[:, 1024:]  = WfT
  - Schedule (timings from the 46.1us baseline trace):
    * ~11 warm-up matmuls on a memset scratch tile run while the input
      streams in, so the PE_HAM clock gate reaches 8/8 (2.4 GHz) before the
      first real matmul - the baseline paid ~1.7us of 1.2 GHz cold matmuls.
    * k-tile 0 is split across both HWDGE rings (x part on sync, W part on
      scalar) so the first real matmul's 384KB dependency becomes two
      concurrent 256/128KB transfers and the PE starts ~1us sooner.
    * Pass 0 (cc half 0, rows m0-m5) runs k-major in lockstep with the
      input stream; at the last k-tile each row's eviction + 3x-replicated
      output store issues immediately, so the output DMA stream starts
      right as the input stream ends (the baseline's all-rows k-major pass
      started stores 11us later, leaving a 10.8us un-overlapped tail).
    * Bridge (cc0, m6-m7) + pass 1 (cc1, all rows) run m-major, one tile
      retiring every ~1.7us, keeping the output ring saturated to the end.
  - Each z tile is written to HBM with a single DMA whose source AP repeats
    the tile 3x (stride-0 middle dim) - the t//3 replication - giving this
    core's contiguous [3072, 1024] slice of the flattened output in bf16.
  - Host unshard = concatenate the 8 slices and upcast to float32.
"""

import json
import os

import numpy as np

import concourse.bass as bass
import concourse.mybir as mybir
from bass_rust import AP
from concourse.tile import TileContext
from concourse.vector_clock import ScopedClock
from concourse.bass_utils import run_bass_kernel_spmd

P = 128          # partitions
C = 1024         # model dim
K_T = C // P     # k tiles
M_T = C // P     # row tiles per core shard
N = 512          # matmul free dim (one PSUM bank of fp32)
L = 3            # replication factor (Tq // Tkv)
ROWS_PER_CORE = 1024
N_CORES = 8
W1 = ROWS_PER_CORE + N   # x | W-cc0 columns per k-tile row block

# compute dtype: "bf16" (half the input DMA), "f32r"/"f32" for debugging
COMPUTE_DT = os.environ.get("KERNEL_COMPUTE_DT", "bf16")
# output dtype on device: "bf16" (host upcasts) or "f32"
OUT_DT = os.environ.get("KERNEL_OUT_DT", "bf16")
# PE warm-up matmuls issued before the first real matmul
N_WARM = int(os.environ.get("KERNEL_N_WARM", "36"))
# k-major lead rows in pass 0
M_LEAD = int(os.environ.get("KERNEL_M_LEAD", "6"))


class SlimTailTileContext(TileContext):
    """Tile's kernel tail is drain -> barrier -> per-semaphore clear
    instructions -> barrier.  The clears only matter if the loaded NEFF
    executes more than once; every kernel() call here builds a fresh jit
    executable (fresh NEFF load, semaphores re-initialized), so skip them
    and the second barrier.  The drain still waits for every DMA queue,
    so outputs are complete before the program ends."""

    def _drain_and_barrier(self, tick_clock, wait_clock):
        drain_inst = self.nc.sync.drain()
        wait_clock.add_sem_waits(
            drain_inst.ins, ScopedClock({None: tick_clock.global_clock})
        )
        popped = self.nc._tile_sem_poison_stack.pop()
        assert popped is self._sem_poison


def _split_multiwaits(nc: bass.Bass) -> None:
    """This container's walrus allows only ONE sync-wait on several
    instruction formats (Drain/CTRL, Matmult's LDWEIGHTS half, ...).  Tile
    can emit more.  Post-pass the serialized BIR: for any instruction with
    >1 on_wait, hoist all but the last wait onto single-wait EventSemaphore
    carriers inserted immediately before it on the same engine (waits then
    execute in queue order - semantics unchanged)."""
    raw = bass.Bass.to_json_bytes(nc)
    j = json.loads(raw)
    for f in j["functions"]:
        for bb in f["blocks"]:
            new_insts = []
            for ins in bb["instructions"]:
                si = ins.get("sync_info")
                waits = si.get("on_wait", []) if si else []
                if len(waits) > 1:
                    for i, w in enumerate(waits[:-1]):
                        carrier = {
                            "engine": ins["engine"],
                            "ins": [],
                            "outs": [],
                            "name": f"{ins['name']}_hw{i}",
                            "opcode": "EventSemaphore",
                            "sync_info": {"on_update": [], "on_wait": [w]},
                        }
                        if "debug" in ins:
                            carrier["debug"] = ins["debug"]
                        new_insts.append(carrier)
                    si["on_wait"] = waits[-1:]
                new_insts.append(ins)
            bb["instructions"] = new_insts
    patched = json.dumps(j).encode()
    nc.to_json_bytes = lambda: patched


def _rep3_src(zh_ap):
    """Source AP reading a [P, N] SBUF tile as [P, L, N] via a stride-0
    middle dim - the DMA replicates each row L times."""
    lay = zh_ap.ap
    assert len(lay) == 2, lay
    return AP(tensor=zh_ap.tensor, offset=zh_ap.offset, ap=[lay[0], [0, L], lay[1]])


def _build(compute_dt: str, out_dt: str) -> bass.Bass:
    nc = bass.Bass("TRN2")
    in_mydt = {
        "bf16": mybir.dt.bfloat16,
        "f32r": mybir.dt.float32r,
        "f32": mybir.dt.float32,
    }[compute_dt]
    out_mydt = {"bf16": mybir.dt.bfloat16, "f32": mybir.dt.float32}[out_dt]

    xw = nc.dram_tensor("xw", [C, 2 * C], in_mydt, kind="ExternalInput")
    out = nc.dram_tensor(
        "out", [L * ROWS_PER_CORE, C], out_mydt, kind="ExternalOutput"
    )
    # out row (L*g + r) <- z row g
    out_rep = out.rearrange("(g r) c -> g r c", r=L)  # [1024, L, 1024]

    with SlimTailTileContext(nc) as tc:
        with (
            tc.tile_pool(name="xw", bufs=1) as xw_pool,
            tc.tile_pool(name="psum", bufs=8, space="PSUM") as psum_pool,
            tc.tile_pool(name="zout", bufs=16) as z_pool,
        ):
            # --- scratch for PE warm-up: memset once, matmul garbage-free
            # zeros into a scratch PSUM bank.  Keeps PE_HAM's activity
            # window busy from the end of the engine preamble (~t+0.7us)
            # until the first real matmul (~t+4us) so the real matmuls run
            # at 2.4 GHz.  N=128 warm-ups are ~110ns cold, so over/under-
            # shooting the real-data arrival costs almost nothing.
            warm = xw_pool.tile([P, P], in_mydt, name="warm", tag="warm")
            nc.vector.memset(warm[:], 0.0)

            # --- input stream.  k0 is split so the very first matmul's
            # dependency is only 96KB (x k0 m0-block on sync, first half of
            # W k0 on scalar); k1..k7 alternate rings; the pass-1-only W
            # half (wc1) follows, split across both rings.
            xwk = []
            for k in range(K_T):
                t = xw_pool.tile([P, W1], in_mydt, name=f"xwk{k}", tag=f"xwk{k}")
                xwk.append(t)
            nc.sync.dma_start(xwk[0][:, :C], xw[0:P, :C])
            nc.scalar.dma_start(xwk[0][:, C:W1], xw[0:P, C:W1])
            in_eng = {1: nc.sync, 3: nc.sync, 5: nc.sync, 7: nc.sync,
                      2: nc.scalar, 4: nc.scalar, 6: nc.scalar}
            for k in range(1, K_T):
                in_eng[k].dma_start(xwk[k][:], xw[k * P : (k + 1) * P, :W1])
            wc1 = xw_pool.tile([P, K_T * N], in_mydt, name="wc1", tag="wc1")
            H = K_T // 2
            nc.sync.dma_start(
                wc1[:, : H * N].rearrange("p (k m) -> p k m", k=H),
                xw[: H * P, W1:].rearrange("(k p) m -> p k m", p=P),
            )
            nc.scalar.dma_start(
                wc1[:, H * N :].rearrange("p (k m) -> p k m", k=H),
                xw[H * P :, W1:].rearrange("(k p) m -> p k m", p=P),
            )

            # --- PE warm-up matmuls (independent of any DMA)
            if N_WARM:
                wps = psum_pool.tile([P, P], mybir.dt.float32, name="wps", tag="ps")
                for i in range(N_WARM):
                    nc.tensor.matmul(
                        wps[:], warm[:], warm[:], start=True, stop=True
                    )

            out_eng = [nc.sync, nc.scalar]
            n_trig = [0]

            def store(zh, m, cc, lo=0, hi=N, eng=None):
                dst = out_rep[m * P : (m + 1) * P, :, cc * N + lo : cc * N + hi]
                src = zh[:, lo:hi]
                if eng is None:
                    eng = out_eng[n_trig[0] % 2]
                    n_trig[0] += 1
                eng.dma_start(dst, _rep3_src(src))

            evict = [nc.vector.tensor_copy, nc.scalar.copy]

            # --- pass 0 (columns 0:512, rows m0..m{M_LEAD-1}): k-major in
            # lockstep with the input stream.  At the last k-tile each row's
            # eviction + store issues immediately after that row's final
            # matmul, so the output stream starts while the PE finishes the
            # remaining rows.
            ps0 = [
                psum_pool.tile([P, N], mybir.dt.float32, name=f"ps0_{m}", tag="ps")
                for m in range(M_LEAD)
            ]
            for k in range(K_T):
                t = xwk[k]
                rhs = t[:, C:W1]
                for m in range(M_LEAD):
                    nc.tensor.matmul(
                        ps0[m][:],
                        t[:, m * P : (m + 1) * P],
                        rhs,
                        start=(k == 0),
                        stop=(k == K_T - 1),
                    )
                    if k == K_T - 1:
                        zh = z_pool.tile([P, N], out_mydt, name=f"z0_{m}", tag="z")
                        evict[m % 2](zh[:], ps0[m][:])
                        store(zh, m, 0)

            # --- bridge (columns 0:512, rows m{M_LEAD}..m7): m-major
            for m in range(M_LEAD, M_T):
                ps = psum_pool.tile([P, N], mybir.dt.float32, name=f"ps0_{m}", tag="ps")
                for k in range(K_T):
                    t = xwk[k]
                    nc.tensor.matmul(
                        ps[:],
                        t[:, m * P : (m + 1) * P],
                        t[:, C:W1],
                        start=(k == 0),
                        stop=(k == K_T - 1),
                    )
                zh = z_pool.tile([P, N], out_mydt, name=f"z0_{m}", tag="z")
                evict[m % 2](zh[:], ps[:])
                store(zh, m, 0)

            # --- pass 1 (columns 512:1024): m-major; the final tile is
            # evicted and stored as two halves on parallel engines to
            # shorten the tail.
            for m in range(M_T):
                ps = psum_pool.tile([P, N], mybir.dt.float32, name=f"ps1_{m}", tag="ps")
                for k in range(K_T):
                    t = xwk[k]
                    nc.tensor.matmul(
                        ps[:],
                        t[:, m * P : (m + 1) * P],
                        wc1[:, k * N : (k + 1) * N],
                        start=(k == 0),
                        stop=(k == K_T - 1),
                    )
                zh = z_pool.tile([P, N], out_mydt, name=f"z1_{m}", tag="z")
                if m == M_T - 1:
                    h = N // 2
                    nc.vector.tensor_copy(zh[:, :h], ps[:, :h])
                    nc.scalar.copy(zh[:, h:], ps[:, h:])
                    store(zh, m, 1, 0, h, eng=nc.sync)
                    store(zh, m, 1, h, N, eng=nc.scalar)
                else:
                    evict[m % 2](zh[:], ps[:])
                    store(zh, m, 1)

    _split_multiwaits(nc)
    return nc


_NC_CACHE: dict = {}


def _get_nc(compute_dt: str, out_dt: str) -> bass.Bass:
    key = (compute_dt, out_dt, N_WARM, M_LEAD)
    if key not in _NC_CACHE:
        _NC_CACHE[key] = _build(compute_dt, out_dt)
    return _NC_CACHE[key]


def kernel(x_q, x_kv, Wq, Wk, Wv, Wproj, _compute_dt=None, _out_dt=None):
    compute_dt = _compute_dt or COMPUTE_DT
    out_dt = _out_dt or OUT_DT
    B, Tkv, C_ = x_kv.shape
    assert (B, Tkv, C_) == (4, 2048, C)

    # Fold the two projections: z = x @ Wv.T @ Wproj.T = x @ WfT
    WfT = (Wv.astype(np.float64).T @ Wproj.astype(np.float64).T).astype(np.float32)

    x_flat = x_kv.reshape(B * Tkv, C)
    in_maps = []
    for c in range(N_CORES):
        shard = x_flat[c * ROWS_PER_CORE : (c + 1) * ROWS_PER_CORE]
        xw = np.concatenate([shard.T, WfT], axis=1)  # [C(k), 2048]
        if compute_dt == "bf16":
            import ml_dtypes

            xw = xw.astype(ml_dtypes.bfloat16)
        else:
            xw = np.ascontiguousarray(xw)
        in_maps.append({"xw": xw})

    nc = _get_nc(compute_dt, out_dt)
    res = run_bass_kernel_spmd(nc, in_maps, core_ids=list(range(N_CORES)))

    Tq = L * Tkv
    blocks = [res.results[c]["out"] for c in range(N_CORES)]
    out_flat = np.concatenate(blocks, axis=0)  # [B*Tq, C]
    return out_flat.reshape(B, Tq, C).astype(np.float32)
